# revision 1
# baseline (speedup 1.0000x reference)
"""Multi-head attention (B=8, N=1024, C=768, H=12) on 8 TRN2 NeuronCores.

Data-parallel over batch: core b computes batch element b end-to-end.

v2: single software-pipelined stream over head PAIRS, built so the ScalarE
exp stream (the hard wall at 96 ACTIVATEs) never starves and the PE never
idles long enough to de-warm the HAM clock gate:

  - ST matmuls of a head pair are row-packed: even head in PE rows 0-63
    (tile_position (0,0)), odd head in rows 64-127 ((64,0)) — the two K=64
    matmuls run concurrently, halving ST time.
  - exp(S*scale - 5) on ScalarE; the -5 shift keeps the unnormalized PV
    outputs and fp16 E in range, and cancels in the normalization.
  - PV keeps the ones-column trick (row 64 = denominator colsum); the
    denominator rows are staged to SBUF and DMA-assembled head-major into
    [8,N]+[4,N] tiles (engine APs need 32-aligned partition bases, DMA
    doesn't), then two batched DVE reciprocals + indicator-matmul
    broadcasts normalize the oT chunks.  The first reciprocal hides under
    the last pairs' exps.
  - qk/v projections and PV/norm of earlier pairs are emitted as "chain
    work" interleaved into each pair's kc loop, filling the PE slack under
    the ScalarE wall instead of running serially before/after attention.

PSUM budget: ST ring 3 x [128,1024] (6 banks) + work ring 2 x [128,512]
(2 banks) = 8 banks.  The work ring hosts 1a/1b accumulations and the PV
accumulators; each chain item allocates and drains atomically so the
2-slot ring can't deadlock.

Host side casts x/weights to fp16 in the layouts above and transposes the
fp32 outT result back.
"""

import os

import numpy as np

import concourse.bass as bass
import concourse.tile as tile
from concourse import mybir
from concourse.bass_utils import run_bass_kernel_spmd

B, N, C = 8, 1024, 768
H, D = 12, 64
NP = H // 2            # head pairs
SCALE = D ** -0.5
EXP_BIAS = -5.0
CC = C // 128          # 6 contraction chunks
NT2 = N // 512         # 2 free-dim chunks of 512
NT8 = N // 128         # 8 partition chunks of 128
F32 = mybir.dt.float32
F16 = mybir.dt.float16

_N_CORES = 8


def _split_multiwaits(nc, max_waits: int = 1):
    """The pinned walrus codegen supports one embedded sync-wait per engine
    instruction (single EVENTS slot in the TPB ISA).  Tile's tail drain /
    barriers accumulate several; hoist all-but-one wait onto same-engine
    NoOps placed immediately before the instruction (waits AND, so order is
    irrelevant)."""
    n_split = 0
    for f in nc.m.functions:
        for blk in f.blocks:
            insts = blk.instructions
            if not any(
                ins.sync_info is not None and len(ins.sync_info.on_wait) > max_waits
                for ins in insts
            ):
                continue
            new_list = []
            for ins in insts:
                si = ins.sync_info
                if si is not None and len(si.on_wait) > max_waits:
                    waits = list(si.on_wait)
                    hoist, keep = waits[:-max_waits], waits[-max_waits:]
                    for w in hoist:
                        nop = mybir.InstNoOp(name=nc.get_next_instruction_name())
                        nop.engine = ins.engine
                        nop.sync_info = mybir.SyncInfo(on_wait=[w], on_update=[])
                        new_list.append(nop)
                        n_split += 1
                    ins.sync_info = mybir.SyncInfo(
                        on_wait=keep, on_update=list(si.on_update)
                    )
                new_list.append(ins)
            blk.instructions = new_list
    return n_split


def _build(split: bool = True):
    nc = bass.Bass()
    xT = nc.declare_dram_parameter("xT", [C, N], F16, isOutput=False)
    wqkT = nc.declare_dram_parameter("wqkT", [C, 2 * C], F16, isOutput=False)
    wvT = nc.declare_dram_parameter("wvT", [C, C], F16, isOutput=False)
    woT = nc.declare_dram_parameter("woT", [C, C], F16, isOutput=False)
    bo = nc.declare_dram_parameter("bo", [C, 1], F32, isOutput=False)
    ind4_d = nc.declare_dram_parameter("ind4", [4, 2 * 128], F16, isOutput=False)
    outT = nc.declare_dram_parameter("outT", [C, N], F32, isOutput=True)

    with tile.TileContext(nc) as tc:
        with (
            tc.tile_pool(name="sb", bufs=1) as sb,
            tc.tile_pool(name="ph2", bufs=29) as ph2,
            tc.tile_pool(name="ph2s", bufs=4) as ph2s,
            tc.tile_pool(name="psum", bufs=1, space="PSUM") as psum,
        ):
            qkT = [
                sb.tile([128, N], F16, tag=f"qkT{j}", name=f"qkT{j}")
                for j in range(2 * CC)
            ]
            v_sb = [
                sb.tile([128, H * (D + 1)], F16, tag=f"v{t}", name=f"v{t}")
                for t in range(NT8)
            ]
            oT = [sb.tile([128, N], F16, tag=f"oT{c}", name=f"oT{c}") for c in range(CC)]
            bo_t = [sb.tile([128, 1], F32, tag=f"bo{c}", name=f"bo{c}") for c in range(CC)]
            xr = [sb.tile([128, N], F16, tag=f"xr{c}", name=f"xr{c}") for c in range(CC)]
            wqk = [
                sb.tile([128, 2 * C], F16, tag=f"wqk{c}", name=f"wqk{c}")
                for c in range(CC)
            ]
            wv = [sb.tile([128, C], F16, tag=f"wv{c}", name=f"wv{c}") for c in range(CC)]
            wo = [sb.tile([128, C], F16, tag=f"wo{c}", name=f"wo{c}") for c in range(CC)]

            # DMA order = first-use order: x (all 1a chunks contract over it),
            # small duplicated wqk j-slices for the lead-in projections (the
            # full wqk tiles arrive later at full-tile DMA efficiency), then
            # wv (1b runs in pair 0's chain slots), wqk, wo.
            for t2 in range(NT2):
                qs = slice(t2 * 512, (t2 + 1) * 512)
                for c in range(CC):
                    sl = slice(c * 128, (c + 1) * 128)
                    nc.sync.dma_start(out=xr[c][:, qs], in_=xT[sl, qs])
            LEAD_J = (0, CC, 1, CC + 1)
            wqks = {}
            for j in LEAD_J:
                js = slice(j * 128, (j + 1) * 128)
                for c in range(CC):
                    sl = slice(c * 128, (c + 1) * 128)
                    t = sb.tile([128, 128], F16, tag=f"wqks{j}_{c}", name=f"wqks{j}_{c}")
                    nc.sync.dma_start(out=t, in_=wqkT[sl, js])
                    wqks[(j, c)] = t
            for c in range(CC):
                sl = slice(c * 128, (c + 1) * 128)
                nc.sync.dma_start(out=wv[c], in_=wvT[sl, :])
            for c in range(CC):
                sl = slice(c * 128, (c + 1) * 128)
                nc.sync.dma_start(out=wqk[c], in_=wqkT[sl, :])
            for c in range(CC):
                sl = slice(c * 128, (c + 1) * 128)
                nc.sync.dma_start(out=wo[c], in_=woT[sl, :])
                nc.sync.dma_start(out=bo_t[c], in_=bo[sl, :])

            ones12 = sb.tile([128, H], F16, tag="ones12")
            nc.vector.memset(ones12, 1.0)
            ebias = sb.tile([128, 1], F32, tag="ebias")
            nc.vector.memset(ebias, EXP_BIAS)

            dallA1 = sb.tile([4, N], F32, tag="dallA1")
            dallA2 = sb.tile([4, N], F32, tag="dallA2")
            dallB = sb.tile([4, N], F32, tag="dallB")
            lnB = sb.tile([4, N], F32, tag="lnB")
            rec16A1 = sb.tile([4, N], F16, tag="rec16A1")
            rec16A2 = sb.tile([4, N], F16, tag="rec16A2")
            rec16B = sb.tile([4, N], F16, tag="rec16B")
            ind4_t = sb.tile([4, 2 * 128], F16, tag="ind4")
            nc.sync.dma_start(out=ind4_t, in_=ind4_d[:, :])

            def st_tile():
                return psum.tile([128, N], F32, tag="st", name="st", bufs=3)

            def ov_tile():
                return psum.tile([128, 512], F32, tag="ov", name="ov", bufs=2)

            # ---------------- emission helpers ----------------
            def emit_1a(j, t2):
                # qkT[j][:, t2-half] = sum_c wqk[c][:, j-block].T @ xr[c][:, t2]
                p = ov_tile()
                for c in range(CC):
                    w = (
                        wqks[(j, c)]
                        if (j, c) in wqks
                        else wqk[c][:, j * 128 : (j + 1) * 128]
                    )
                    nc.tensor.matmul(
                        p,
                        w,
                        xr[c][:, t2 * 512 : (t2 + 1) * 512],
                        start=(c == 0),
                        stop=(c == CC - 1),
                    )
                nc.vector.tensor_copy(qkT[j][:, t2 * 512 : (t2 + 1) * 512], p)

            def emit_1b(t8, nh):
                # v[t8][:, h*65:h*65+64] = sum_c xr[c][:, t8].T @ wv[c]
                p = ov_tile()
                for c in range(CC):
                    nc.tensor.matmul(
                        p[:, 0:384],
                        xr[c][:, t8 * 128 : (t8 + 1) * 128],
                        wv[c][:, nh * 384 : (nh + 1) * 384],
                        start=(c == 0),
                        stop=(c == CC - 1),
                    )
                v_view = v_sb[t8].rearrange("p (h e) -> p h e", e=D + 1)
                nc.vector.tensor_copy(
                    v_view[:, nh * 6 : (nh + 1) * 6, 0:D],
                    p[:, 0:384].rearrange("p (h d) -> p h d", d=D),
                )
                if nh == 1:
                    nc.vector.tensor_copy(
                        v_view[:, :, D : D + 1], ones12.unsqueeze(2)
                    )

            exps = {}

            def emit_st_pair(p_, kc):
                # row-packed: even head in PE rows 0-63, odd head in 64-127
                se = st_tile()
                so = st_tile()
                kb = slice(kc * 128, (kc + 1) * 128)
                for t2 in range(NT2):
                    sl = slice(t2 * 512, (t2 + 1) * 512)
                    nc.tensor.matmul(
                        se[:, sl], qkT[CC + p_][0:D, kb], qkT[p_][0:D, sl],
                        start=True, stop=True,
                    )
                    nc.tensor.matmul(
                        so[:, sl], qkT[CC + p_][D:128, kb], qkT[p_][D:128, sl],
                        start=True, stop=True,
                    )
                ee = ph2.tile([128, N], F16, tag="exps", name="exps")
                eo = ph2.tile([128, N], F16, tag="exps", name="exps")
                nc.scalar.activation(
                    ee, se, mybir.ActivationFunctionType.Exp, scale=SCALE, bias=ebias
                )
                nc.scalar.activation(
                    eo, so, mybir.ActivationFunctionType.Exp, scale=SCALE, bias=ebias
                )
                exps[(2 * p_, kc)] = ee
                exps[(2 * p_ + 1, kc)] = eo

            pv_state = {}

            def pv_mm_range(h, ovs, kcs):
                for kc in kcs:
                    e = exps.pop((h, kc))
                    for t2 in range(NT2):
                        nc.tensor.matmul(
                            ovs[t2][0 : D + 1, :],
                            v_sb[kc][:, h * (D + 1) : (h + 1) * (D + 1)],
                            e[:, t2 * 512 : (t2 + 1) * 512],
                            start=(kc == 0),
                            stop=(kc == NT8 - 1),
                        )

            def emit_pv_mms(h):
                ovs = [ov_tile() for _ in range(NT2)]
                pv_mm_range(h, ovs, range(NT8))
                return ovs

            def pv_q(h, q):
                # quarter-granularity PV chain items (4 MMs each) so a PV
                # burst never exceeds one slot's PE budget; q=0 allocates,
                # q=3 drains.  Parts of one head must stay adjacent in the
                # chain list (ov-ring WAR deadlock otherwise).
                if q == 0:
                    ovs = [ov_tile() for _ in range(NT2)]
                    pv_state[h] = ovs
                    pv_mm_range(h, ovs, range(2))
                else:
                    pv_mm_range(h, pv_state[h], range(2 * q, 2 * q + 2))
                    if q == 3:
                        emit_pv_drain(h, pv_state.pop(h))

            def ipv(h):
                return [lambda q=q: pv_q(h, q) for q in range(4)]

            def pv_open(h):
                # first half of the PV accumulation; holds both ov-ring
                # slots until pv_close — no other ov-allocating item may be
                # emitted in between (ring WAR would deadlock the PE FIFO)
                ovs = [ov_tile() for _ in range(NT2)]
                pv_state[h] = ovs
                pv_mm_range(h, ovs, range(NT8 // 2))

            def pv_close(h):
                ovs = pv_state.pop(h)
                pv_mm_range(h, ovs, range(NT8 // 2, NT8))
                emit_pv_drain(h, ovs)

            def emit_pv_drain(h, ovs, stg_first=False):
                po = (h % 2) * 64
                dtile, row = (
                    (dallA1, h) if h < 4
                    else (dallA2, h - 4) if h < 8
                    else (dallB, h - 8)
                )
                for t2 in range(NT2):
                    o = ovs[t2]
                    qs = slice(t2 * 512, (t2 + 1) * 512)
                    if not stg_first:
                        nc.vector.tensor_copy(oT[h // 2][po : po + D, qs], o[0:D, :])
                    stg = ph2s.tile([1, 512], F32, tag="stg", name="stg")
                    nc.vector.tensor_copy(stg, o[D : D + 1, :])
                    nc.sync.dma_start(out=dtile[row : row + 1, qs], in_=stg)
                if stg_first:
                    for t2 in range(NT2):
                        qs = slice(t2 * 512, (t2 + 1) * 512)
                        nc.vector.tensor_copy(
                            oT[h // 2][po : po + D, qs], ovs[t2][0:D, :]
                        )

            def emit_pv_full(h):
                emit_pv_drain(h, emit_pv_mms(h))

            def emit_recip(dtile, r16tile):
                # fp16 output is plenty for a softmax denominator (5e-4 rel)
                with nc.allow_low_precision("softmax denom recip to fp16"):
                    nc.vector.reciprocal(r16tile, dtile)

            def emit_r_chunk(r16tile, i, c):
                ind_t = ind4_t
                ps = [ov_tile(), ov_tile()]
                for t2 in range(NT2):
                    nc.tensor.matmul(
                        ps[t2],
                        ind_t[:, i * 128 : (i + 1) * 128],
                        r16tile[:, t2 * 512 : (t2 + 1) * 512],
                        start=True,
                        stop=True,
                    )
                rr = ph2s.tile([128, N], F16, tag="r16", name="r16", bufs=2)
                for t2 in range(NT2):
                    nc.vector.tensor_copy(rr[:, t2 * 512 : (t2 + 1) * 512], ps[t2])
                nc.vector.tensor_mul(oT[c], oT[c], rr)

            # ---------------- lead-in: just enough qk for ST(0,0) ---------
            for j, t2 in ((0, 0), (0, 1), (CC, 0)):
                emit_1a(j, t2)

            # ---------------- attention: global slot stream ---------------
            # STs are emitted two slots ahead of their slot's chain items so
            # a multi-us chain item never delays the next ACT; chain items
            # are kept fine (<= ~2us).
            def i1a(j, t2):
                return lambda: emit_1a(j, t2)

            def i1b(t8, nh):
                return lambda: emit_1b(t8, nh)

            chains = {p_: [] for p_ in range(NP)}
            chains[0] = [i1a(CC, 1), i1a(1, 0), i1a(1, 1), i1a(CC + 1, 0),
                         i1a(CC + 1, 1)]
            chains[0] += [i1b(t8, nh) for t8 in range(6) for nh in range(2)]
            chains[1] = [i1b(t8, nh) for t8 in (6, 7) for nh in range(2)]
            for p_ in (1, 2, 3, 4):
                for j in (p_ + 1, CC + p_ + 1):
                    for t2 in range(NT2):
                        chains[p_].append(i1a(j, t2))
            chains[1] += [*ipv(0),
                          *ipv(1)]
            chains[2] += [*ipv(2),
                          *ipv(3)]
            # the batched reciprocals are 8us of DVE each: schedule them
            # where the DVE queue has slack so they never delay a PV drain
            # (a delayed drain stalls the ov ring -> PV backlog -> the last
            # STs block in the PE FIFO -> ScalarE starves; that cascade cost
            # v4/v6 ~25us)
            chains[3] = ([lambda: emit_recip(dallA1, rec16A1)] + chains[3]
                         + [*ipv(4),
                            *ipv(5)])
            chains[4] += [*ipv(6),
                          *ipv(7),
                          lambda: emit_r_chunk(rec16A1, 0, 0),
                          lambda: emit_r_chunk(rec16A1, 1, 1),
                          lambda: emit_recip(dallA2, rec16A2)]
            chains[5] = [
                lambda: pv_open(8), lambda: pv_close(8),
                lambda: pv_open(9), lambda: pv_close(9),
                lambda: emit_r_chunk(rec16A2, 0, 2),
                lambda: emit_r_chunk(rec16A2, 1, 3),
            ]

            slots = [(p_, kc) for p_ in range(NP) for kc in range(NT8)]
            slot_items = [[] for _ in slots]
            for p_ in range(NP - 1):
                items = chains[p_]
                done = 0
                for kc in range(NT8):
                    want = (len(items) * (kc + 1) + NT8 - 1) // NT8
                    while done < want:
                        slot_items[p_ * NT8 + kc].append(items[done])
                        done += 1

            # pair 5 is pinned by hand: every ov-ring user must be emitted
            # before pv10 opens (it holds both slots until the tail drain),
            # and pv10's kc MMs trail its ACTs by ~2 kc so the accumulation
            # hides under the last exps instead of running post-stream
            ovs10 = []

            def pv10_start():
                ovs10.extend(ov_tile() for _ in range(NT2))
                pv_mm_range(10, ovs10, range(4))

            def pv10_mm(kc):
                return lambda: pv_mm_range(10, ovs10, (kc,))

            c5 = chains[5]
            for kc, items in enumerate((
                [c5[0]], [c5[1]], [c5[2]], [c5[3]], [c5[4]],
                [c5[5], pv10_start], [pv10_mm(4), pv10_mm(5)], [pv10_mm(6)],
            )):
                slot_items[5 * NT8 + kc].extend(items)

            LOOKAHEAD = 2
            for i in range(LOOKAHEAD):
                emit_st_pair(*slots[i])
            for i in range(len(slots)):
                if i + LOOKAHEAD < len(slots):
                    emit_st_pair(*slots[i + LOOKAHEAD])
                for item in slot_items[i]:
                    item()

            # ---------------- tail ----------------------------------------
            # pv11 on the PE while ScalarE computes 1/dallB as exp(-ln d)
            # (the table-set swap + both activations hide under pv11); the
            # first three output-projection accumulations open with their
            # c=0..3 contributions (already normalized) so the PE never
            # idles >3.4us and phase 3 runs at the warm clock.
            pv_mm_range(10, ovs10, (7,))
            emit_pv_drain(10, ovs10, stg_first=True)
            ovs11 = emit_pv_mms(11)
            emit_pv_drain(11, ovs11, stg_first=True)
            nc.scalar.activation(lnB, dallB, mybir.ActivationFunctionType.Ln)
            with nc.allow_low_precision("softmax denom recip to fp16"):
                nc.scalar.activation(
                    rec16B, lnB, mybir.ActivationFunctionType.Exp, scale=-1.0
                )

            with tc.tile_pool(name="ph3o", bufs=3) as ph3o:

                def ph3_mms(p, fc, cs, start):
                    for c in cs:
                        for t2 in range(NT2):
                            nc.tensor.matmul(
                                p[:, t2 * 512 : (t2 + 1) * 512],
                                wo[c][:, fc * 128 : (fc + 1) * 128],
                                oT[c][:, t2 * 512 : (t2 + 1) * 512],
                                start=(start and c == cs[0]),
                                stop=(c == CC - 1),
                            )

                def ph3_finish(p, fc):
                    ph3_mms(p, fc, (4, 5), False)
                    ot = ph3o.tile([128, N], F32, tag="outsb", name="outsb")
                    nc.vector.tensor_scalar_add(ot, p, bo_t[fc])
                    nc.sync.dma_start(
                        out=outT[fc * 128 : (fc + 1) * 128, :], in_=ot
                    )

                ph3_ps = []
                for fc in range(3):
                    p = st_tile()
                    ph3_mms(p, fc, (0, 1, 2, 3), True)
                    ph3_ps.append(p)
                emit_r_chunk(rec16B, 0, 4)
                emit_r_chunk(rec16B, 1, 5)
                for fc in range(3):
                    ph3_finish(ph3_ps[fc], fc)
                for fc in range(3, CC):
                    p = st_tile()
                    ph3_mms(p, fc, (0, 1, 2, 3), True)
                    ph3_finish(p, fc)

    if split:
        _split_multiwaits(nc)
    return nc


_NC = None


def _get_nc():
    global _NC
    if _NC is None:
        _NC = _build()
    return _NC


def kernel(x, w_qkv, w_out, b_out):
    x = np.asarray(x, dtype=np.float32)
    w_qkv = np.asarray(w_qkv, dtype=np.float32)
    w_out = np.asarray(w_out, dtype=np.float32)
    b_out = np.asarray(b_out, dtype=np.float32)

    wqkT = np.ascontiguousarray(w_qkv[: 2 * C].T.astype(np.float16))
    wvT = np.ascontiguousarray(w_qkv[2 * C :].T.astype(np.float16))
    woT = np.ascontiguousarray(w_out.T.astype(np.float16))
    bo = np.ascontiguousarray(b_out.reshape(C, 1))
    ind4 = np.zeros((4, 2 * 128), dtype=np.float16)
    for c in range(2):
        ind4[2 * c, c * 128 : c * 128 + D] = 1.0
        ind4[2 * c + 1, c * 128 + D : (c + 1) * 128] = 1.0

    in_maps = [
        {
            "xT": np.ascontiguousarray(x[b].T.astype(np.float16)),
            "wqkT": wqkT,
            "wvT": wvT,
            "woT": woT,
            "bo": bo,
            "ind4": ind4,
        }
        for b in range(B)
    ]

    nc = _get_nc()
    trace = bool(os.environ.get("KERNEL_TRACE"))
    res = run_bass_kernel_spmd(nc, in_maps, list(range(_N_CORES)), trace=trace)
    if trace:
        print(f"HW exec time: {res.exec_time_ns} ns")
        if res.instructions_and_trace is not None:
            print(f"trace: {res.instructions_and_trace[1]}")

    out = np.empty((B, N, C), dtype=np.float32)
    for b in range(B):
        out[b] = res.results[b]["outT"].T
    return out

